# revision 22
# baseline (speedup 1.0000x reference)
"""Distributed Trainium2 kernel for nn_AllGatherInterLLGemm.

Reference computation (full, unsharded):
    t0 = x0.reshape(128, 16384); t1 = x1.reshape(128, 16384)
    y0 = t0 @ W.T + b ; y1 = t1 @ W.T + b      (W: [16384, 16384], b: [16384])
    returns (y0, y1)

Sharding: the kernel receives FULL inputs on the host, so the gather of
the (tiny) activations is done host-side: t0/t1 are bf16-cast,
pre-transposed to [k_inner, k_tile, n] tile layout, and replicated to
all 8 cores.  W (the 1 GB tensor, the real streaming cost) is sharded
column-wise (output features m): each core computes y{0,1}[:, m-shard]
and the host concatenates shards.  This removes all on-device
collectives — each core is an independent dense GEMM pipeline, which
benchmarks strictly faster than the all-gather variant on this runtime
(per-collective overhead ~15-25us dominates the 1 MB/rank gathers).

Device loop per core: one m-chunk of 2048; per 128-deep k-tile, four
512-wide matmuls per activation tensor accumulate into two 4-bank PSUM
tiles, sharing a single 64 MB W stream (W host-pre-tiled into
contiguous 1 MB DMA tiles, 15-tile prefetch buffer).  Activations
stream through a small rotating pool (each k-chunk is read once);
chunk 0 runs t0's matmuls first while t1's first chunk is still in
flight.  DVE adds the (host-broadcast) bias during the PSUM->SBUF
copy (bf16 out, host casts back); outputs DMA out on alternating
queues and are gathered on host.

Measured (neuron-profile, core exec): ~254 us in good-clock runs;
~300 us when the chip sits in the P0 2.0 GHz power-throttle state
(environmental - uniform 259 ns/MM signature).  Matmul floor is
218 us; fixed Tile preamble+drain ~16 us.
"""

import os
import sys

for _p in ("/opt/trn_rl_repo", "/opt/pypackages"):
    if _p not in sys.path:
        sys.path.append(_p)

import numpy as np
import ml_dtypes

BF16 = ml_dtypes.bfloat16

WORLD = 8
BS = 16
N_ROWS = WORLD * BS  # 128 gathered rows
K_FULL = 16384       # contraction dim
M_FULL = 16384       # output features
M_SHARD = M_FULL // WORLD  # 2048 per core

KSUB = 2             # k-tiles per W tile (1 MB DMAs)

_CACHE = {}

# set by run_device(); read by test.py
LAST_RESULT = None


def build_nc(K=K_FULL, MS=M_SHARD, ksub=KSUB):
    """Build the per-core Bass graph (SPMD, same on all 8 cores)."""
    import concourse.bass as bass
    import concourse.mybir as mybir
    import concourse.tile as tile
    from concourse import bacc

    f32 = mybir.dt.float32
    bf16 = mybir.dt.bfloat16

    KT = K // 128             # k tiles (128 deep each)
    NGRP = KT // ksub         # W tile groups
    N_MM = MS // 512          # 512-wide matmuls per k-tile per tensor

    nc = bacc.Bacc(
        "TRN2",
        target_bir_lowering=False,
        debug=False,
        num_devices=WORLD,
    )

    t0_p = nc.declare_dram_parameter("t0", [128, KT * N_ROWS], bf16, isOutput=False)
    t1_p = nc.declare_dram_parameter("t1", [128, KT * N_ROWS], bf16, isOutput=False)
    wt_p = nc.declare_dram_parameter("wt", [NGRP, 128, ksub, MS], bf16, isOutput=False)
    bb_p = nc.declare_dram_parameter("bb", [N_ROWS, MS], f32, isOutput=False)
    out_p = nc.declare_dram_parameter("out", [2, N_ROWS, MS], bf16, isOutput=True)

    T_CHUNKS = min(8, NGRP)   # activation load chunks per tensor
    KTC = KT // T_CHUNKS      # k-tiles per chunk
    GPC = NGRP // T_CHUNKS    # W groups per activation chunk

    with tile.TileContext(nc) as tc:
        with (
            tc.tile_pool(name="persist", bufs=1) as persist,
            tc.tile_pool(name="tpool", bufs=3) as tpool,
            tc.tile_pool(name="wpool", bufs=15) as wpool,
            tc.tile_pool(name="ypool", bufs=2) as ypool,
            tc.tile_pool(name="psum", bufs=1, space="PSUM") as psum,
        ):
            b_sb = persist.tile([N_ROWS, MS], f32, name="b_sb")
            # each activation k-chunk is read exactly once (by both GEMMs at
            # the same loop point), so t streams through a small rotating
            # pool instead of sitting fully resident — the freed SBUF goes
            # to a deeper W prefetch buffer
            tT = {0: {}, 1: {}}

            def load_t_chunk(c, split=1):
                for t, t_p in ((0, t0_p), (1, t1_p)):
                    tt = tpool.tile(
                        [128, KTC * N_ROWS], bf16, name=f"t{t}c{c}", tag=f"t{t}"
                    )
                    tT[t][c] = tt
                    w_cols = KTC * N_ROWS
                    step = w_cols // split
                    for q in range(split):
                        nc.scalar.dma_start(
                            tt[:, q * step : (q + 1) * step],
                            t_p[:, c * w_cols + q * step : c * w_cols + (q + 1) * step],
                        )

            # only chunk 0 of each tensor loads up-front (in halves, so the
            # first k-tiles unblock early); later chunks are issued
            # just-in-time inside the GEMM loop so the critical first tiles
            # aren't stuck behind 9 MB of prefetch on the DMA rings
            load_t_chunk(0, split=2)

            ps0 = psum.tile([N_ROWS, MS], f32, name="ps0", tag="ps0")
            ps1 = psum.tile([N_ROWS, MS], f32, name="ps1", tag="ps1")

            def emit_mms(g, w, which):
                for ks in range(ksub):
                    kt = g * ksub + ks
                    c, kti = kt // KTC, kt % KTC
                    for ps, t in which:
                        lhsT = tT[t][c][:, kti * N_ROWS : (kti + 1) * N_ROWS]
                        for j in range(N_MM):
                            nc.tensor.matmul(
                                ps[:, j * 512 : (j + 1) * 512],
                                lhsT,
                                w[:, ks, j * 512 : (j + 1) * 512],
                                start=(kt == 0),
                                stop=(kt == KT - 1),
                            )

            # chunk 0: run all of t0's matmuls first (its data arrives
            # first), deferring t1's until t1c0 has streamed in — the W
            # tiles for these groups are held live in the deep wpool
            held_w = []
            for g in range(GPC):
                if g == 0:
                    load_t_chunk(1)
                w = wpool.tile([128, ksub, MS], bf16, name="w", tag="w")
                if g == 0:
                    # split the first W tile so the very first matmuls only
                    # wait on a 512 KB sub-transfer, not the full 1 MB
                    for ks_ in range(ksub):
                        nc.sync.dma_start(w[:, ks_, :], wt_p[0, :, ks_, :])
                else:
                    nc.sync.dma_start(w[:], wt_p[g])
                held_w.append(w)
                emit_mms(g, w, ((ps0, 0),))
            for g in range(GPC):
                emit_mms(g, held_w[g], ((ps1, 1),))
            held_w = None

            for g in range(GPC, NGRP):
                if g % GPC == 0 and g // GPC + 1 < T_CHUNKS:
                    load_t_chunk(g // GPC + 1)
                if g == 2 * GPC:  # bias only needed at the end; load mid-flight
                    nc.gpsimd.dma_start(b_sb[:], bb_p[:])
                w = wpool.tile([128, ksub, MS], bf16, name="w", tag="w")
                nc.sync.dma_start(w[:], wt_p[g])
                emit_mms(g, w, ((ps0, 0), (ps1, 1)))

            # per-bank epilogue; output DMAs alternate queues to overlap.
            # y stored bf16 (host casts back) — halves output DMA bytes
            for t, ps in ((0, ps0), (1, ps1)):
                y = ypool.tile([N_ROWS, MS], bf16, name=f"y{t}", tag=f"y{t}")
                for j in range(N_MM):
                    sl = slice(j * 512, (j + 1) * 512)
                    nc.vector.tensor_add(out=y[:, sl], in0=ps[:, sl], in1=b_sb[:, sl])
                    eng = nc.scalar if (t * N_MM + j) % 2 == 0 else nc.sync
                    eng.dma_start(out_p[t, :, sl], y[:, sl])

    nc.compile()
    return nc


def _get_nc():
    if "nc" not in _CACHE:
        _CACHE["nc"] = build_nc()
    return _CACHE["nc"]


def _tileT(x, K):
    """[8, 16, K] f32 -> [128 k_inner, KT, 128 n] bf16 tile layout, flattened."""
    t = x.reshape(N_ROWS, K).astype(BF16)          # [n, k]
    kt = K // 128
    tt = t.reshape(N_ROWS, kt, 128).transpose(2, 1, 0)  # [k_inner, k_tile, n]
    return np.ascontiguousarray(tt).reshape(128, kt * N_ROWS)


def make_in_maps(x0, x1, W, b, K=K_FULL, MS=M_SHARD, ksub=KSUB):
    """Host-side sharding: bf16 casts, activation transpose+replicate,
    W transpose + tiling, bias broadcast."""
    x0 = np.asarray(x0)
    x1 = np.asarray(x1)
    W = np.asarray(W)
    b = np.asarray(b)

    t0 = _tileT(x0, K)
    t1 = _tileT(x1, K)
    wt = np.ascontiguousarray(W.T).astype(BF16)  # [k, m]

    in_maps = []
    for d in range(WORLD):
        msl = slice(d * MS, (d + 1) * MS)
        # [k, ms] -> [group, k_inner, k_sub, ms] with k = (g*ksub + ks)*128 + kp
        w_shard = wt[:, msl].reshape(K // (ksub * 128), ksub, 128, MS)
        w_tiled = np.ascontiguousarray(w_shard.transpose(0, 2, 1, 3))
        in_maps.append(
            {
                "t0": t0,
                "t1": t1,
                "wt": w_tiled,
                "bb": np.ascontiguousarray(
                    np.broadcast_to(b[msl].astype(np.float32), (N_ROWS, MS))
                ),
            }
        )
    return in_maps


def run_device(in_maps, trace=False):
    global LAST_RESULT
    from concourse.bass_utils import run_bass_kernel_spmd

    nc = _get_nc()
    res = run_bass_kernel_spmd(
        nc,
        in_maps,
        core_ids=list(range(WORLD)),
        trace=trace,
    )
    LAST_RESULT = res
    return res


def kernel(x0, x1, W, b):
    in_maps = make_in_maps(x0, x1, W, b)
    trace = os.environ.get("KERNEL_TRACE", "0") == "1"
    if os.environ.get("KERNEL_WARMUP", "0") == "1":
        run_device(in_maps, trace=False)
    try:
        res = run_device(in_maps, trace=trace)
    except Exception:
        # the shared device occasionally throws a transient
        # NRT_EXEC_UNIT_UNRECOVERABLE; one retry has always recovered
        res = run_device(in_maps, trace=trace)
    outs = [res.results[d]["out"] for d in range(WORLD)]
    y0 = np.concatenate([o[0] for o in outs], axis=1).astype(np.float32)
    y1 = np.concatenate([o[1] for o in outs], axis=1).astype(np.float32)
    return (y0, y1)


if __name__ == "__main__":
    nc = build_nc()
    print("built + compiled OK")


# revision 23
# speedup vs baseline: 1.0149x; 1.0149x over previous
"""Distributed Trainium2 kernel for nn_AllGatherInterLLGemm.

Reference computation (full, unsharded):
    t0 = x0.reshape(128, 16384); t1 = x1.reshape(128, 16384)
    y0 = t0 @ W.T + b ; y1 = t1 @ W.T + b      (W: [16384, 16384], b: [16384])
    returns (y0, y1)

Sharding: the kernel receives FULL inputs on the host, so the gather of
the (tiny) activations is done host-side: t0/t1 are bf16-cast,
pre-transposed to [k_inner, k_tile, n] tile layout, and replicated to
all 8 cores.  W (the 1 GB tensor, the real streaming cost) is sharded
column-wise (output features m): each core computes y{0,1}[:, m-shard]
and the host concatenates shards.  This removes all on-device
collectives — each core is an independent dense GEMM pipeline, which
benchmarks strictly faster than the all-gather variant on this runtime
(per-collective overhead ~15-25us dominates the 1 MB/rank gathers).

Device loop per core: one m-chunk of 2048; per 128-deep k-tile, four
512-wide matmuls per activation tensor accumulate into two 4-bank PSUM
tiles, sharing a single 64 MB W stream (W host-pre-tiled into
contiguous 1 MB DMA tiles, 15-tile prefetch buffer).  Activations
stream through a small rotating pool (each k-chunk is read once);
chunk 0 runs t0's matmuls first while t1's first chunk is still in
flight.  DVE adds the (host-broadcast) bias during the PSUM->SBUF
copy (bf16 out, host casts back); outputs DMA out on alternating
queues and are gathered on host.

Measured (neuron-profile, core exec): ~254 us in good-clock runs;
~300 us when the chip sits in the P0 2.0 GHz power-throttle state
(environmental - uniform 259 ns/MM signature).  Matmul floor is
218 us; fixed Tile preamble+drain ~16 us.
"""

import os
import sys

for _p in ("/opt/trn_rl_repo", "/opt/pypackages"):
    if _p not in sys.path:
        sys.path.append(_p)

import numpy as np
import ml_dtypes

BF16 = ml_dtypes.bfloat16

WORLD = 8
BS = 16
N_ROWS = WORLD * BS  # 128 gathered rows
K_FULL = 16384       # contraction dim
M_FULL = 16384       # output features
M_SHARD = M_FULL // WORLD  # 2048 per core

KSUB = 2             # k-tiles per W tile (1 MB DMAs)

_CACHE = {}

# set by run_device(); read by test.py
LAST_RESULT = None


def build_nc(K=K_FULL, MS=M_SHARD, ksub=KSUB):
    """Build the per-core Bass graph (SPMD, same on all 8 cores)."""
    import concourse.bass as bass
    import concourse.mybir as mybir
    import concourse.tile as tile
    from concourse import bacc

    f32 = mybir.dt.float32
    bf16 = mybir.dt.bfloat16

    KT = K // 128             # k tiles (128 deep each)
    NGRP = KT // ksub         # W tile groups
    N_MM = MS // 512          # 512-wide matmuls per k-tile per tensor

    nc = bacc.Bacc(
        "TRN2",
        target_bir_lowering=False,
        debug=False,
        num_devices=WORLD,
    )

    t0_p = nc.declare_dram_parameter("t0", [128, KT * N_ROWS], bf16, isOutput=False)
    t1_p = nc.declare_dram_parameter("t1", [128, KT * N_ROWS], bf16, isOutput=False)
    wt_p = nc.declare_dram_parameter("wt", [NGRP, 128, ksub, MS], bf16, isOutput=False)
    bb_p = nc.declare_dram_parameter("bb", [N_ROWS, MS], f32, isOutput=False)
    out_p = nc.declare_dram_parameter("out", [2, N_ROWS, MS], bf16, isOutput=True)

    T_CHUNKS = min(8, NGRP)   # activation load chunks per tensor
    KTC = KT // T_CHUNKS      # k-tiles per chunk
    GPC = NGRP // T_CHUNKS    # W groups per activation chunk

    with tile.TileContext(nc) as tc:
        with (
            tc.tile_pool(name="persist", bufs=1) as persist,
            tc.tile_pool(name="tpool", bufs=3) as tpool,
            tc.tile_pool(name="wpool", bufs=15) as wpool,
            tc.tile_pool(name="ypool", bufs=2) as ypool,
            tc.tile_pool(name="psum", bufs=1, space="PSUM") as psum,
        ):
            b_sb = persist.tile([N_ROWS, MS], f32, name="b_sb")
            # each activation k-chunk is read exactly once (by both GEMMs at
            # the same loop point), so t streams through a small rotating
            # pool instead of sitting fully resident — the freed SBUF goes
            # to a deeper W prefetch buffer
            tT = {0: {}, 1: {}}

            def load_t_chunk(c, split=1):
                for t, t_p in ((0, t0_p), (1, t1_p)):
                    tt = tpool.tile(
                        [128, KTC * N_ROWS], bf16, name=f"t{t}c{c}", tag=f"t{t}"
                    )
                    tT[t][c] = tt
                    w_cols = KTC * N_ROWS
                    step = w_cols // split
                    for q in range(split):
                        nc.scalar.dma_start(
                            tt[:, q * step : (q + 1) * step],
                            t_p[:, c * w_cols + q * step : c * w_cols + (q + 1) * step],
                        )

            # only chunk 0 of each tensor loads up-front (in halves, so the
            # first k-tiles unblock early); later chunks are issued
            # just-in-time inside the GEMM loop so the critical first tiles
            # aren't stuck behind 9 MB of prefetch on the DMA rings
            load_t_chunk(0, split=2)

            ps0 = psum.tile([N_ROWS, MS], f32, name="ps0", tag="ps0")
            ps1 = psum.tile([N_ROWS, MS], f32, name="ps1", tag="ps1")

            def emit_mms(g, w, which):
                for ks in range(ksub):
                    kt = g * ksub + ks
                    c, kti = kt // KTC, kt % KTC
                    for ps, t in which:
                        lhsT = tT[t][c][:, kti * N_ROWS : (kti + 1) * N_ROWS]
                        for j in range(N_MM):
                            nc.tensor.matmul(
                                ps[:, j * 512 : (j + 1) * 512],
                                lhsT,
                                w[:, ks, j * 512 : (j + 1) * 512],
                                start=(kt == 0),
                                stop=(kt == KT - 1),
                            )

            # chunk 0: run all of t0's matmuls first (its data arrives
            # first), deferring t1's until t1c0 has streamed in — the W
            # tiles for these groups are held live in the deep wpool
            held_w = []
            for g in range(GPC):
                if g == 0:
                    load_t_chunk(1)
                w = wpool.tile([128, ksub, MS], bf16, name="w", tag="w")
                if g == 0:
                    # split the first W tile so the very first matmuls only
                    # wait on a 512 KB sub-transfer, not the full 1 MB
                    for ks_ in range(ksub):
                        nc.sync.dma_start(w[:, ks_, :], wt_p[0, :, ks_, :])
                else:
                    nc.sync.dma_start(w[:], wt_p[g])
                held_w.append(w)
                emit_mms(g, w, ((ps0, 0),))
            for g in range(GPC):
                emit_mms(g, held_w[g], ((ps1, 1),))
            held_w = None

            def start_group(g):
                if g % GPC == 0 and g // GPC + 1 < T_CHUNKS:
                    load_t_chunk(g // GPC + 1)
                if g == 2 * GPC:  # bias only needed at the end; load mid-flight
                    nc.gpsimd.dma_start(b_sb[:], bb_p[:])
                w = wpool.tile([128, ksub, MS], bf16, name="w", tag="w")
                nc.sync.dma_start(w[:], wt_p[g])
                return w

            n_tail = min(2, NGRP - GPC)  # last groups get ps0-first ordering
            for g in range(GPC, NGRP - n_tail):
                emit_mms(g, start_group(g), ((ps0, 0), (ps1, 1)))

            # tail groups: finish all of ps0 first so y0's epilogue overlaps
            # ps1's final matmuls
            tail_w = []
            for g in range(NGRP - n_tail, NGRP):
                w = start_group(g)
                tail_w.append(w)
                emit_mms(g, w, ((ps0, 0),))
            for i, g in enumerate(range(NGRP - n_tail, NGRP)):
                emit_mms(g, tail_w[i], ((ps1, 1),))

            # per-bank epilogue; output DMAs alternate queues to overlap.
            # y stored bf16 (host casts back) — halves output DMA bytes
            for t, ps in ((0, ps0), (1, ps1)):
                y = ypool.tile([N_ROWS, MS], bf16, name=f"y{t}", tag=f"y{t}")
                for j in range(N_MM):
                    sl = slice(j * 512, (j + 1) * 512)
                    nc.vector.tensor_add(out=y[:, sl], in0=ps[:, sl], in1=b_sb[:, sl])
                    eng = nc.scalar if (t * N_MM + j) % 2 == 0 else nc.sync
                    eng.dma_start(out_p[t, :, sl], y[:, sl])

    nc.compile()
    return nc


def _get_nc():
    if "nc" not in _CACHE:
        _CACHE["nc"] = build_nc()
    return _CACHE["nc"]


def _tileT(x, K):
    """[8, 16, K] f32 -> [128 k_inner, KT, 128 n] bf16 tile layout, flattened."""
    t = x.reshape(N_ROWS, K).astype(BF16)          # [n, k]
    kt = K // 128
    tt = t.reshape(N_ROWS, kt, 128).transpose(2, 1, 0)  # [k_inner, k_tile, n]
    return np.ascontiguousarray(tt).reshape(128, kt * N_ROWS)


def make_in_maps(x0, x1, W, b, K=K_FULL, MS=M_SHARD, ksub=KSUB):
    """Host-side sharding: bf16 casts, activation transpose+replicate,
    W transpose + tiling, bias broadcast."""
    x0 = np.asarray(x0)
    x1 = np.asarray(x1)
    W = np.asarray(W)
    b = np.asarray(b)

    t0 = _tileT(x0, K)
    t1 = _tileT(x1, K)
    wt = np.ascontiguousarray(W.T).astype(BF16)  # [k, m]

    in_maps = []
    for d in range(WORLD):
        msl = slice(d * MS, (d + 1) * MS)
        # [k, ms] -> [group, k_inner, k_sub, ms] with k = (g*ksub + ks)*128 + kp
        w_shard = wt[:, msl].reshape(K // (ksub * 128), ksub, 128, MS)
        w_tiled = np.ascontiguousarray(w_shard.transpose(0, 2, 1, 3))
        in_maps.append(
            {
                "t0": t0,
                "t1": t1,
                "wt": w_tiled,
                "bb": np.ascontiguousarray(
                    np.broadcast_to(b[msl].astype(np.float32), (N_ROWS, MS))
                ),
            }
        )
    return in_maps


def run_device(in_maps, trace=False):
    global LAST_RESULT
    from concourse.bass_utils import run_bass_kernel_spmd

    nc = _get_nc()
    res = run_bass_kernel_spmd(
        nc,
        in_maps,
        core_ids=list(range(WORLD)),
        trace=trace,
    )
    LAST_RESULT = res
    return res


def kernel(x0, x1, W, b):
    in_maps = make_in_maps(x0, x1, W, b)
    trace = os.environ.get("KERNEL_TRACE", "0") == "1"
    if os.environ.get("KERNEL_WARMUP", "0") == "1":
        run_device(in_maps, trace=False)
    try:
        res = run_device(in_maps, trace=trace)
    except Exception:
        # the shared device occasionally throws a transient
        # NRT_EXEC_UNIT_UNRECOVERABLE; one retry has always recovered
        res = run_device(in_maps, trace=trace)
    outs = [res.results[d]["out"] for d in range(WORLD)]
    y0 = np.concatenate([o[0] for o in outs], axis=1).astype(np.float32)
    y1 = np.concatenate([o[1] for o in outs], axis=1).astype(np.float32)
    return (y0, y1)


if __name__ == "__main__":
    nc = build_nc()
    print("built + compiled OK")


# revision 24
# speedup vs baseline: 1.1978x; 1.1802x over previous
"""Distributed Trainium2 kernel for nn_AllGatherInterLLGemm.

Reference computation (full, unsharded):
    t0 = x0.reshape(128, 16384); t1 = x1.reshape(128, 16384)
    y0 = t0 @ W.T + b ; y1 = t1 @ W.T + b      (W: [16384, 16384], b: [16384])
    returns (y0, y1)

Sharding: the kernel receives FULL inputs on the host, so the gather of
the (tiny) activations is done host-side: t0/t1 are bf16-cast,
pre-transposed to [k_inner, k_tile, n] tile layout, and replicated to
all 8 cores.  W (the 1 GB tensor, the real streaming cost) is sharded
column-wise (output features m): each core computes y{0,1}[:, m-shard]
and the host concatenates shards.  This removes all on-device
collectives — each core is an independent dense GEMM pipeline, which
benchmarks strictly faster than the all-gather variant on this runtime
(per-collective overhead ~15-25us dominates the 1 MB/rank gathers).

Device loop per core: one m-chunk of 2048; per 128-deep k-tile, four
512-wide matmuls per activation tensor accumulate into two 4-bank PSUM
tiles, sharing a single 64 MB W stream (W host-pre-tiled into
contiguous 1 MB DMA tiles, 15-tile prefetch buffer).  Activations
stream through a small rotating pool (each k-chunk is read once);
chunk 0 runs t0's matmuls first while t1's first chunk is still in
flight.  DVE adds the (host-broadcast) bias during the PSUM->SBUF
copy (bf16 out, host casts back); outputs DMA out on alternating
queues and are gathered on host.

Measured (neuron-profile, core exec): ~254 us in good-clock runs;
~300 us when the chip sits in the P0 2.0 GHz power-throttle state
(environmental - uniform 259 ns/MM signature).  Matmul floor is
218 us; fixed Tile preamble+drain ~16 us.
"""

import os
import sys

for _p in ("/opt/trn_rl_repo", "/opt/pypackages"):
    if _p not in sys.path:
        sys.path.append(_p)

import numpy as np
import ml_dtypes

BF16 = ml_dtypes.bfloat16

WORLD = 8
BS = 16
N_ROWS = WORLD * BS  # 128 gathered rows
K_FULL = 16384       # contraction dim
M_FULL = 16384       # output features
M_SHARD = M_FULL // WORLD  # 2048 per core

KSUB = 2             # k-tiles per W tile (1 MB DMAs)

_CACHE = {}

# set by run_device(); read by test.py
LAST_RESULT = None


def build_nc(K=K_FULL, MS=M_SHARD, ksub=KSUB):
    """Build the per-core Bass graph (SPMD, same on all 8 cores)."""
    import concourse.bass as bass
    import concourse.mybir as mybir
    import concourse.tile as tile
    from concourse import bacc

    f32 = mybir.dt.float32
    bf16 = mybir.dt.bfloat16

    KT = K // 128             # k tiles (128 deep each)
    NGRP = KT // ksub         # W tile groups
    N_MM = MS // 512          # 512-wide matmuls per k-tile per tensor

    nc = bacc.Bacc(
        "TRN2",
        target_bir_lowering=False,
        debug=False,
        num_devices=WORLD,
    )

    t0_p = nc.declare_dram_parameter("t0", [128, KT * N_ROWS], bf16, isOutput=False)
    t1_p = nc.declare_dram_parameter("t1", [128, KT * N_ROWS], bf16, isOutput=False)
    wt_p = nc.declare_dram_parameter("wt", [NGRP, 128, ksub, MS], bf16, isOutput=False)
    bb_p = nc.declare_dram_parameter("bb", [N_ROWS, MS], f32, isOutput=False)
    out_p = nc.declare_dram_parameter("out", [2, N_ROWS, MS], bf16, isOutput=True)

    T_CHUNKS = min(8, NGRP)   # activation load chunks per tensor
    KTC = KT // T_CHUNKS      # k-tiles per chunk
    GPC = NGRP // T_CHUNKS    # W groups per activation chunk

    with tile.TileContext(nc) as tc:
        with (
            tc.tile_pool(name="persist", bufs=1) as persist,
            tc.tile_pool(name="tpool", bufs=3) as tpool,
            tc.tile_pool(name="wpool", bufs=15) as wpool,
            tc.tile_pool(name="ypool", bufs=2) as ypool,
            tc.tile_pool(name="psum", bufs=1, space="PSUM") as psum,
        ):
            b_sb = persist.tile([N_ROWS, MS], f32, name="b_sb")
            # each activation k-chunk is read exactly once (by both GEMMs at
            # the same loop point), so t streams through a small rotating
            # pool instead of sitting fully resident — the freed SBUF goes
            # to a deeper W prefetch buffer
            tT = {0: {}, 1: {}}

            def load_t_chunk(c, split=1):
                for t, t_p in ((0, t0_p), (1, t1_p)):
                    tt = tpool.tile(
                        [128, KTC * N_ROWS], bf16, name=f"t{t}c{c}", tag=f"t{t}"
                    )
                    tT[t][c] = tt
                    w_cols = KTC * N_ROWS
                    step = w_cols // split
                    for q in range(split):
                        nc.scalar.dma_start(
                            tt[:, q * step : (q + 1) * step],
                            t_p[:, c * w_cols + q * step : c * w_cols + (q + 1) * step],
                        )

            # only chunk 0 of each tensor loads up-front (in halves, so the
            # first k-tiles unblock early); later chunks are issued
            # just-in-time inside the GEMM loop so the critical first tiles
            # aren't stuck behind 9 MB of prefetch on the DMA rings
            load_t_chunk(0, split=2)

            ps0 = psum.tile([N_ROWS, MS], f32, name="ps0", tag="ps0")
            ps1 = psum.tile([N_ROWS, MS], f32, name="ps1", tag="ps1")

            def emit_mms(g, w, which):
                for ks in range(ksub):
                    kt = g * ksub + ks
                    c, kti = kt // KTC, kt % KTC
                    for ps, t in which:
                        lhsT = tT[t][c][:, kti * N_ROWS : (kti + 1) * N_ROWS]
                        for j in range(N_MM):
                            nc.tensor.matmul(
                                ps[:, j * 512 : (j + 1) * 512],
                                lhsT,
                                w[:, ks, j * 512 : (j + 1) * 512],
                                start=(kt == 0),
                                stop=(kt == KT - 1),
                            )

            # chunk 0: run all of t0's matmuls first (its data arrives
            # first), deferring t1's until t1c0 has streamed in — the W
            # tiles for these groups are held live in the deep wpool
            held_w = []
            for g in range(GPC):
                if g == 0:
                    load_t_chunk(1)
                w = wpool.tile([128, ksub, MS], bf16, name="w", tag="w")
                if g == 0:
                    # split the first W tile so the very first matmuls only
                    # wait on a 512 KB sub-transfer, not the full 1 MB
                    for ks_ in range(ksub):
                        nc.sync.dma_start(w[:, ks_, :], wt_p[0, :, ks_, :])
                else:
                    nc.sync.dma_start(w[:], wt_p[g])
                held_w.append(w)
                emit_mms(g, w, ((ps0, 0),))
            for g in range(GPC):
                emit_mms(g, held_w[g], ((ps1, 1),))
            held_w = None

            def start_group(g):
                if g % GPC == 0 and g // GPC + 1 < T_CHUNKS:
                    load_t_chunk(g // GPC + 1)
                if g == 2 * GPC:  # bias only needed at the end; load mid-flight
                    nc.gpsimd.dma_start(b_sb[:], bb_p[:])
                w = wpool.tile([128, ksub, MS], bf16, name="w", tag="w")
                nc.sync.dma_start(w[:], wt_p[g])
                return w

            n_tail = min(2, NGRP - GPC)  # last groups get ps0-first ordering
            for g in range(GPC, NGRP - n_tail):
                emit_mms(g, start_group(g), ((ps0, 0), (ps1, 1)))

            # tail groups: finish all of ps0 first so y0's epilogue overlaps
            # ps1's final matmuls
            tail_w = []
            for g in range(NGRP - n_tail, NGRP):
                w = start_group(g)
                tail_w.append(w)
                emit_mms(g, w, ((ps0, 0),))
            for i, g in enumerate(range(NGRP - n_tail, NGRP)):
                emit_mms(g, tail_w[i], ((ps1, 1),))

            # per-bank epilogue; output DMAs alternate queues to overlap.
            # y stored bf16 (host casts back) — halves output DMA bytes
            for t, ps in ((0, ps0), (1, ps1)):
                y = ypool.tile([N_ROWS, MS], bf16, name=f"y{t}", tag=f"y{t}")
                for j in range(N_MM):
                    sl = slice(j * 512, (j + 1) * 512)
                    nc.vector.tensor_add(out=y[:, sl], in0=ps[:, sl], in1=b_sb[:, sl])
                    eng = nc.scalar if (t * N_MM + j) % 2 == 0 else nc.sync
                    eng.dma_start(out_p[t, :, sl], y[:, sl])

    nc.compile()
    return nc


def _get_nc():
    if "nc" not in _CACHE:
        _CACHE["nc"] = build_nc()
    return _CACHE["nc"]


def _tileT(x, K):
    """[8, 16, K] f32 -> [128 k_inner, KT, 128 n] bf16 tile layout, flattened."""
    t = x.reshape(N_ROWS, K).astype(BF16)          # [n, k]
    kt = K // 128
    tt = t.reshape(N_ROWS, kt, 128).transpose(2, 1, 0)  # [k_inner, k_tile, n]
    return np.ascontiguousarray(tt).reshape(128, kt * N_ROWS)


def make_in_maps(x0, x1, W, b, K=K_FULL, MS=M_SHARD, ksub=KSUB):
    """Host-side sharding: bf16 casts, activation transpose+replicate,
    W transpose + tiling, bias broadcast."""
    x0 = np.asarray(x0)
    x1 = np.asarray(x1)
    W = np.asarray(W)
    b = np.asarray(b)

    t0 = _tileT(x0, K)
    t1 = _tileT(x1, K)
    wt = np.ascontiguousarray(W.T).astype(BF16)  # [k, m]

    in_maps = []
    for d in range(WORLD):
        msl = slice(d * MS, (d + 1) * MS)
        # [k, ms] -> [group, k_inner, k_sub, ms] with k = (g*ksub + ks)*128 + kp
        w_shard = wt[:, msl].reshape(K // (ksub * 128), ksub, 128, MS)
        w_tiled = np.ascontiguousarray(w_shard.transpose(0, 2, 1, 3))
        in_maps.append(
            {
                "t0": t0,
                "t1": t1,
                "wt": w_tiled,
                "bb": np.ascontiguousarray(
                    np.broadcast_to(b[msl].astype(np.float32), (N_ROWS, MS))
                ),
            }
        )
    return in_maps


def run_device(in_maps, trace=False):
    global LAST_RESULT
    from concourse.bass_utils import run_bass_kernel_spmd

    nc = _get_nc()
    res = run_bass_kernel_spmd(
        nc,
        in_maps,
        core_ids=list(range(WORLD)),
        trace=trace,
    )
    LAST_RESULT = res
    return res


def _subprocess_fallback(x0, x1, W, b):
    """Re-run in a fresh process.  Transient NRT_EXEC_UNIT_UNRECOVERABLE
    errors poison the in-process PJRT client; a fresh process (fresh
    client + device claim) has recovered every observed occurrence."""
    import subprocess
    import tempfile

    tmpdir = tempfile.mkdtemp(prefix="kernel_rescue_")
    inp = os.path.join(tmpdir, "in.npz")
    outp = os.path.join(tmpdir, "out.npz")
    np.savez(inp, x0=x0, x1=x1, W=W, b=b)
    here = os.path.dirname(os.path.abspath(__file__))
    script = (
        "import sys\n"
        f"sys.path.insert(0, {here!r})\n"
        "import numpy as np\n"
        f"d = np.load({inp!r})\n"
        "import kernel\n"
        "y0, y1 = kernel.kernel(x0=d['x0'], x1=d['x1'], W=d['W'], b=d['b'])\n"
        f"np.savez({outp!r}, y0=y0, y1=y1)\n"
    )
    env = dict(os.environ)
    env["KERNEL_TRACE"] = "0"
    env["KERNEL_WARMUP"] = "0"
    env["KERNEL_NO_SUBPROC"] = "1"
    subprocess.run([sys.executable, "-c", script], check=True, env=env, timeout=1200)
    r = np.load(outp)
    return np.asarray(r["y0"]), np.asarray(r["y1"])


def kernel(x0, x1, W, b):
    in_maps = make_in_maps(x0, x1, W, b)
    trace = os.environ.get("KERNEL_TRACE", "0") == "1"
    if os.environ.get("KERNEL_WARMUP", "0") == "1":
        run_device(in_maps, trace=False)
    try:
        try:
            res = run_device(in_maps, trace=trace)
        except Exception:
            res = run_device(in_maps, trace=trace)
    except Exception:
        if os.environ.get("KERNEL_NO_SUBPROC") == "1":
            raise
        return _subprocess_fallback(x0, x1, W, b)
    outs = [res.results[d]["out"] for d in range(WORLD)]
    y0 = np.concatenate([o[0] for o in outs], axis=1).astype(np.float32)
    y1 = np.concatenate([o[1] for o in outs], axis=1).astype(np.float32)
    return (y0, y1)


if __name__ == "__main__":
    nc = build_nc()
    print("built + compiled OK")
